# revision 1
# baseline (speedup 1.0000x reference)
"""Trainium2 Bass kernel for the BoSs decoder layer (self-contained).

Sharding (8 cores, tensor-parallel):
  - Attention: 2 query heads + their 1 KV head per core; o-proj partial sums.
  - MLP: 1024 of 8192 intermediate rows per core; down-proj partial sums.
  - Cross-core partial sums are reduced on host between/after two launches.
  - RMSNorm is folded on host: the kernel inputs are the pre-normalized
    activations in bf16 (norm weights are folded into the projection
    weights, as is the 1/sqrt(d) attention scale).

Attention runs entirely in the "transposed score" (S^T = K Q^T) layout:
  - x^T / y^T are transposed on the host (xbar transpose-DMA works but
    serializes globally against all regular DMA traffic, which starves the
    weight streams), so the kernel only does plain streaming loads.
  - scores are built per 128-wide key block directly in [k, q] layout, so
    P^T (the PV moving operand) comes straight out of the exp with no
    transposes. Row sums are recovered with a ones-vector matmul and the
    normalization is applied to O^T via a broadcast reciprocal.
  - the segment mask is a rank-4 matmul (+32768 where sid matches, then
    exp(x - 32768)); causal masking skips above-diagonal key blocks
    entirely and handles the two diagonal blocks with small triangular
    mask matmuls (exact in bf16).
  - both query heads are processed together: q chunks are 256 wide and the
    two heads ride side by side in the 512-wide moving dimension, so mask
    matmuls and row sums are shared between heads.
"""

import sys

if "/opt/trn_rl_repo" not in sys.path:
    sys.path.insert(0, "/opt/trn_rl_repo")

from contextlib import ExitStack

import ml_dtypes
import numpy as np

import concourse.bass as bass
import concourse.mybir as mybir
import concourse.tile as tile
from concourse.bass_utils import run_bass_kernel_spmd

F32 = mybir.dt.float32
BF16 = mybir.dt.bfloat16
AF = mybir.ActivationFunctionType
ALU = mybir.AluOpType

HEADS = 16
KV_HEADS = 8
D = 128          # head dim
H = 2048         # hidden
INTER = 8192
NSTATE = 4
EPS = 1e-6
THETA = 10000.0
S = 2048         # sequence length
NC = 8           # cores
NEG = -32768.0   # additive mask magnitude; exact in bf16 and f32

QH = HEADS // NC          # 2 query heads / core
MI = INTER // NC // 128   # 8 inter chunks of 128 / core
NCH = S // 512            # 4 column chunks
NC8 = S // 256            # 8 quarter chunks (two heads share a 512 lane)
NHC = H // 128            # 16 hidden chunks
NKB = S // 128            # 16 key blocks


def _patched_drain_and_barrier(self, tick_clock, wait_clock):
    # This walrus build supports only ONE sync wait per Drain instruction;
    # split the TileContext tail drain's waits across single-wait drains.
    drain_inst = self.nc.sync.drain()
    wait_clock.add_sem_waits(
        drain_inst.ins, tile.ScopedClock({None: tick_clock.global_clock})
    )
    si = drain_inst.ins.sync_info
    waits = list(si.on_wait) if si and si.on_wait else []
    if len(waits) > 1:
        drain_inst.ins.sync_info = mybir.SyncInfo(
            on_wait=[waits[0]], on_update=list(si.on_update)
        )
        for w in waits[1:]:
            d2 = self.nc.sync.drain()
            d2.ins.sync_info = mybir.SyncInfo(on_wait=[w], on_update=[])
    self.nc.all_engine_barrier()
    assert self.sems is not None
    popped = self.nc._tile_sem_poison_stack.pop()
    assert popped is self._sem_poison
    self.nc.clear_and_free_semaphores(list(self.sems.allocated().values()))
    self.nc.all_engine_barrier()


tile.TileContext._drain_and_barrier = _patched_drain_and_barrier


def _split_multi_waits(j):
    """Walrus in this env encodes at most ONE sync wait per instruction.
    Tile attaches several. Split: insert single-wait EventSemaphore
    instructions on the same engine immediately before the instruction."""
    ctr = 0
    for f in j["functions"]:
        for bb in f["blocks"]:
            insts = bb["instructions"]
            if not any(
                len(((i.get("sync_info") or {}).get("on_wait") or [])) > 1
                for i in insts
            ):
                continue
            new_insts = []
            for inst in insts:
                si = inst.get("sync_info")
                waits = (si or {}).get("on_wait") or []
                if len(waits) > 1:
                    for w in waits[:-1]:
                        ctr += 1
                        new_insts.append({
                            "debug": inst.get("debug"),
                            "engine": inst["engine"],
                            "ins": [],
                            "outs": [],
                            "name": f"{inst['name']}_sw{ctr}",
                            "opcode": "EventSemaphore",
                            "sync_info": {"on_update": [], "on_wait": [w]},
                        })
                    si["on_wait"] = [waits[-1]]
                new_insts.append(inst)
            bb["instructions"] = new_insts
    return j


_orig_to_json_bytes = bass.Bass.to_json_bytes


def _to_json_bytes_split(self):
    import json as _json

    j = _json.loads(_orig_to_json_bytes(self))
    _split_multi_waits(j)
    return _json.dumps(j).encode()


bass.Bass.to_json_bytes = _to_json_bytes_split


def build_attn():
    nc = bass.Bass()
    xnT = nc.dram_tensor("xnT", [H, S], BF16, kind="ExternalInput")
    wq = nc.dram_tensor("wq", [128, NHC, QH * D], BF16, kind="ExternalInput")
    wk = nc.dram_tensor("wk", [128, NHC, D], BF16, kind="ExternalInput")
    wv = nc.dram_tensor("wv", [128, NHC, D], BF16, kind="ExternalInput")
    wo = nc.dram_tensor("wo", [128, QH, H], BF16, kind="ExternalInput")
    cosT = nc.dram_tensor("cosT", [128, S], BF16, kind="ExternalInput")
    sinT = nc.dram_tensor("sinT", [128, S], BF16, kind="ExternalInput")
    NM = sum(2 * c8 + 2 for c8 in range(NC8))   # 72 mask tiles
    m01 = nc.dram_tensor("m01", [NM, 128, 512], BF16, kind="ExternalInput")
    oA = nc.dram_tensor("oA", [S, H], BF16, kind="ExternalOutput")

    with tile.TileContext(nc) as tc, ExitStack() as ctx:
        consts = ctx.enter_context(tc.tile_pool(name="consts", bufs=1))

        from concourse.masks import make_identity
        ident = consts.tile([128, 128], BF16)
        make_identity(nc, ident)
        ones_bf = consts.tile([128, 1], BF16)
        nc.vector.memset(ones_bf, 1.0)
        ones_row = consts.tile([1, 128], BF16)
        nc.vector.memset(ones_row, 1.0)
        wq_sb = consts.tile([128, NHC, QH * D], BF16)
        nc.sync.dma_start(out=wq_sb, in_=wq[:, :, :])
        wk_sb = consts.tile([128, NHC, D], BF16)
        nc.sync.dma_start(out=wk_sb, in_=wk[:, :, :])
        wv_sb = consts.tile([128, NHC, D], BF16)
        nc.sync.dma_start(out=wv_sb, in_=wv[:, :, :])
        wo_sb = consts.tile([128, QH, H], BF16)
        cos_sb = consts.tile([128, S], BF16)
        nc.sync.dma_start(out=cos_sb, in_=cosT[:, :])
        sin_sb = consts.tile([128, S], BF16)
        nc.sync.dma_start(out=sin_sb, in_=sinT[:, :])
        nc.sync.dma_start(out=wo_sb, in_=wo[:, :, :])

        qT_all = consts.tile([128, QH, S], BF16)   # [d, h, s]
        kT_all = consts.tile([128, S], BF16)       # [d, s]
        vsb = consts.tile([128, NKB, D], BF16)     # [k % 128, k // 128, d]

        # ---- phase 1: host-transposed input + projections + rope --------
        with ExitStack() as ph1:
            big = ph1.enter_context(tc.tile_pool(name="big", bufs=1))
            xnT_sb = [big.tile([128, S], BF16, name=f"xnT{b}")
                      for b in range(NHC)]
            for b in range(NHC):
                nc.scalar.dma_start(out=xnT_sb[b],
                                    in_=xnT[b * 128:(b + 1) * 128, :])
            ps_proj = ph1.enter_context(
                tc.tile_pool(name="psP1", bufs=7, space="PSUM"))
            ps_T = ph1.enter_context(
                tc.tile_pool(name="psT1", bufs=1, space="PSUM"))
            rope_pool = ph1.enter_context(tc.tile_pool(name="rope", bufs=2))

            def rope(ps, sl, out_ap):
                t1 = rope_pool.tile([128, 512], F32, tag="r1")
                nc.vector.tensor_mul(t1, ps, cos_sb[:, sl])
                t2 = rope_pool.tile([128, 512], F32, tag="r2")
                nc.vector.tensor_mul(t2[0:64], ps[64:128, :],
                                     sin_sb[0:64, sl])
                nc.vector.tensor_mul(t2[64:128], ps[0:64, :],
                                     sin_sb[64:128, sl])
                nc.vector.tensor_add(out_ap, t1, t2)

            def postprocess(t, ci, ps):
                sl = slice(ci * 512, (ci + 1) * 512)
                if t == "v":
                    vT_sb = rope_pool.tile([128, 512], BF16, tag="vT")
                    nc.vector.tensor_copy(vT_sb, ps)
                    pstv = ps_T.tile([128, 512], BF16, tag="psT")
                    for j in range(4):
                        nc.tensor.transpose(
                            pstv[:, j * 128:(j + 1) * 128],
                            vT_sb[:, j * 128:(j + 1) * 128], ident)
                    nc.scalar.copy(
                        vsb[:, ci * 4:(ci + 1) * 4, :],
                        pstv.rearrange("p (c f) -> p c f", c=4))
                elif t == "k":
                    rope(ps, sl, kT_all[:, sl])
                else:
                    rope(ps, sl, qT_all[:, 0 if t == "q0" else 1, sl])

            # flat task cascade, 6-wide at the start so the PE has work
            # while the xnT wave streams in
            tasks = [(t, ci) for ci in range(NCH)
                     for t in ("q0", "q1", "k", "v")]
            groups = [tasks[0:7], tasks[7:11], tasks[11:14], tasks[14:16]]
            for gi, grp in enumerate(groups):
                pss = [ps_proj.tile([128, 512], F32, tag="psP",
                                    name=f"psp_{gi}_{i}")
                       for i in range(len(grp))]
                for hc in range(NHC):
                    st = (hc == 0)
                    sp = (hc == NHC - 1)
                    for i, (t, ci) in enumerate(grp):
                        sl = slice(ci * 512, (ci + 1) * 512)
                        if t == "q0":
                            lhs = wq_sb[:, hc, 0:D]
                        elif t == "q1":
                            lhs = wq_sb[:, hc, D:2 * D]
                        elif t == "k":
                            lhs = wk_sb[:, hc, :]
                        else:
                            lhs = wv_sb[:, hc, :]
                        nc.tensor.matmul(pss[i], lhs, xnT_sb[hc][:, sl],
                                         start=st, stop=sp)
                # v first: its PE transposes only wait on one short DVE copy
                order = sorted(range(len(grp)),
                               key=lambda i: grp[i][0] != "v")
                for i in order:
                    t, ci = grp[i]
                    postprocess(t, ci, pss[i])

        # ---- phase 2: S^T-layout attention + o-proj ---------------------
        with ExitStack() as ph2:
            pt_pool = ph2.enter_context(tc.tile_pool(name="pt", bufs=2))
            mk_pool = ph2.enter_context(tc.tile_pool(name="mk", bufs=4))
            ot_pool = ph2.enter_context(tc.tile_pool(name="ot", bufs=2))
            out_pool = ph2.enter_context(tc.tile_pool(name="out", bufs=2))
            st_pool = ph2.enter_context(tc.tile_pool(name="ast", bufs=4))
            ps_S = ph2.enter_context(
                tc.tile_pool(name="psS", bufs=2, space="PSUM"))
            ps_R = ph2.enter_context(
                tc.tile_pool(name="psR", bufs=1, space="PSUM"))
            ps_O = ph2.enter_context(
                tc.tile_pool(name="psO", bufs=1, space="PSUM"))
            ps_P = ph2.enter_context(
                tc.tile_pool(name="psP", bufs=2, space="PSUM"))

            def oproj(c8, oTn):
                for qb in range(2):
                    qi = c8 * 2 + qb
                    outsb = out_pool.tile([128, H], BF16, tag="out")
                    for hc4 in range(4):
                        sl = slice(hc4 * 512, (hc4 + 1) * 512)
                        psP = ps_P.tile([128, 512], F32, tag="psP")
                        for h in range(QH):
                            nc.tensor.matmul(
                                psP, oTn[:, h, qb * 128:(qb + 1) * 128],
                                wo_sb[:, h, sl],
                                start=(h == 0), stop=(h == QH - 1))
                        if hc4 % 2 == 0:
                            nc.scalar.copy(outsb[:, sl], psP)
                        else:
                            nc.vector.tensor_copy(outsb[:, sl], psP)
                        if hc4 == 1:
                            nc.sync.dma_start(
                                out=oA[qi * 128:(qi + 1) * 128, 0:1024],
                                in_=outsb[:, 0:1024])
                    nc.sync.dma_start(
                        out=oA[qi * 128:(qi + 1) * 128, 1024:2048],
                        in_=outsb[:, 1024:2048])

            mi_base = {}
            _mi = 0
            for c8 in range(NC8):
                mi_base[c8] = _mi
                _mi += 2 * c8 + 2
            pending = None                         # (c8, oTn) one chunk late
            # chunk 1 first (its kT/qT deps are ready before the projection
            # tail), then big-to-small so the kernel ends on cheap chunks
            for c8 in [1, 7, 6, 5, 4, 3, 2, 0]:
                mi = mi_base[c8]
                nb = 2 * c8 + 2
                qsl = slice(c8 * 256, (c8 + 1) * 256)
                qmov = qT_all[:, :, qsl]           # [d, 2, 256] moving
                strip = pt_pool.tile([128, NKB, 512], BF16, tag="strip")
                mstrip = pt_pool.tile([128, NKB, 512], BF16, tag="mstrip")
                for g in range(nb // 2):
                    b0 = 2 * g
                    psS = ps_S.tile([128, 1024], F32, tag="psS")
                    for b in (b0, b0 + 1):
                        ksl = slice(b * 128, (b + 1) * 128)
                        off = (b - b0) * 512
                        nc.tensor.matmul(psS[:, off:off + 512],
                                         kT_all[:, ksl], qmov,
                                         start=True, stop=True)
                    nc.scalar.activation(
                        strip[:, b0:b0 + 2, :],
                        psS.rearrange("p (c f) -> p c f", c=2), AF.Exp)
                    mt = mk_pool.tile([128, 2, 512], BF16, tag="m01")
                    nc.sync.dma_start(
                        out=mt, in_=m01[mi:mi + 2].rearrange("c p f -> p c f"))
                    mi += 2
                    # alternate the mask multiply between the DVE and the
                    # otherwise-idle gpsimd so neither paces the exp chain
                    eng = nc.gpsimd if g % 2 == 0 else nc.vector
                    eng.tensor_tensor(mstrip[:, b0:b0 + 2, :],
                                      strip[:, b0:b0 + 2, :], mt,
                                      op=ALU.mult)
                if pending is not None:
                    oproj(*pending)
                psR = ps_R.tile([1, 512], F32, tag="psR")
                for b in range(nb):
                    nc.tensor.matmul(psR, ones_bf, mstrip[:, b, :],
                                     start=(b == 0), stop=(b == nb - 1))
                sums_sb = st_pool.tile([1, 512], BF16, tag="sums")
                nc.vector.tensor_copy(sums_sb, psR)
                psB = ps_S.tile([128, 512], F32, tag="psS")
                nc.tensor.matmul(psB, ones_row, sums_sb,
                                 start=True, stop=True)
                rb = st_pool.tile([128, 512], F32, tag="rb")
                nc.vector.reciprocal(rb, psB)
                psO = ps_O.tile([128, 512], F32, tag="psO")
                for b in range(nb):
                    nc.tensor.matmul(psO, vsb[:, b, :], mstrip[:, b, :],
                                     start=(b == 0), stop=(b == nb - 1))
                oTn = ot_pool.tile([128, QH, 256], BF16, tag="oTn")
                nc.vector.tensor_tensor(
                    oTn[:, :, :], psO.rearrange("p (h f) -> p h f", h=QH),
                    rb.rearrange("p (h f) -> p h f", h=QH), op=ALU.mult)
                pending = (c8, oTn)
            oproj(*pending)
    return nc


def build_mlp():
    nc = bass.Bass()
    ynT = nc.dram_tensor("ynT", [H, S], BF16, kind="ExternalInput")
    wg = nc.dram_tensor("wg", [MI, 128, NHC, 128], BF16, kind="ExternalInput")
    wu = nc.dram_tensor("wu", [MI, 128, NHC, 128], BF16, kind="ExternalInput")
    wd = nc.dram_tensor("wd", [128, MI, H], BF16, kind="ExternalInput")
    oB = nc.dram_tensor("oB", [S, H], BF16, kind="ExternalOutput")

    with tile.TileContext(nc) as tc, ExitStack() as ctx:
        consts = ctx.enter_context(tc.tile_pool(name="consts", bufs=1))
        ps_gu = ctx.enter_context(
            tc.tile_pool(name="psGU", bufs=6, space="PSUM"))
        ps_d = ctx.enter_context(
            tc.tile_pool(name="psD", bufs=2, space="PSUM"))

        wsl_pool = ctx.enter_context(tc.tile_pool(name="wsl", bufs=2))
        sg_pool = ctx.enter_context(tc.tile_pool(name="sg", bufs=2))
        out_pool = ctx.enter_context(tc.tile_pool(name="out", bufs=2))

        # first gate/up weights ahead of everything so PE starts immediately
        wgu_first = []
        for m in range(1):
            wg_sb = wsl_pool.tile([128, NHC, 128], BF16, tag="wg")
            nc.sync.dma_start(out=wg_sb, in_=wg[m])
            wu_sb = wsl_pool.tile([128, NHC, 128], BF16, tag="wu")
            nc.sync.dma_start(out=wu_sb, in_=wu[m])
            wgu_first.append((wg_sb, wu_sb))
        ynT_sb = [consts.tile([128, S], BF16, name=f"ynT{b}")
                  for b in range(NHC)]
        for b in range(NHC):
            nc.scalar.dma_start(out=ynT_sb[b],
                                in_=ynT[b * 128:(b + 1) * 128, :])
        wd_sb = consts.tile([128, MI, H], BF16)
        mT_ch = [consts.tile([128, MI, 512], BF16, tag=f"mT_{i}",
                             name=f"mT_{i}")
                 for i in range(NCH)]

        for m in range(MI):
            if m < len(wgu_first):
                wg_sb, wu_sb = wgu_first[m]
            else:
                wg_sb = wsl_pool.tile([128, NHC, 128], BF16, tag="wg")
                nc.sync.dma_start(out=wg_sb, in_=wg[m])
                wu_sb = wsl_pool.tile([128, NHC, 128], BF16, tag="wu")
                nc.sync.dma_start(out=wu_sb, in_=wu[m])
            if m == 3:
                # after the early gate/up weight stream so it doesn't stall
                # the m=1..2 loads; still ~200us ahead of the down phase
                nc.sync.dma_start(out=wd_sb, in_=wd[:, :, :])
            ci_groups = ([(0, 1, 2), (3,)] if m == 0
                         else [(0, 1), (2, 3)])
            for cis in ci_groups:
                # interleave (g,u) x chunks over hc; 6 live accumulations
                # for the first pass chase the incoming ynT DMA wave
                ps4 = [ps_gu.tile([128, 512], F32, tag="psGU",
                                  name=f"gu_{m}_{cis[0]}_{i}")
                       for i in range(2 * len(cis))]
                for hc in range(NHC):
                    st_ = (hc == 0)
                    sp_ = (hc == NHC - 1)
                    for i, ci in enumerate(cis):
                        sl = slice(ci * 512, (ci + 1) * 512)
                        nc.tensor.matmul(ps4[2 * i], wg_sb[:, hc, :],
                                         ynT_sb[hc][:, sl],
                                         start=st_, stop=sp_)
                        nc.tensor.matmul(ps4[2 * i + 1], wu_sb[:, hc, :],
                                         ynT_sb[hc][:, sl],
                                         start=st_, stop=sp_)
                for i, ci in enumerate(cis):
                    sg = sg_pool.tile([128, 512], BF16, tag="sg")
                    nc.scalar.activation(sg, ps4[2 * i], AF.Silu)
                    nc.vector.tensor_tensor(mT_ch[ci][:, m, :], sg,
                                            ps4[2 * i + 1], op=ALU.mult)

        for st in range(S // 128):
            ssl = slice((st % 4) * 128, (st % 4) * 128 + 128)
            outsb = out_pool.tile([128, H], BF16, tag="out")
            for ci in range(H // 512):
                sl = slice(ci * 512, (ci + 1) * 512)
                psd = ps_d.tile([128, 512], F32, tag="psD")
                for m in range(MI):
                    nc.tensor.matmul(psd, mT_ch[st // 4][:, m, ssl],
                                     wd_sb[:, m, sl],
                                     start=(m == 0), stop=(m == MI - 1))
                if ci % 2 == 0:
                    nc.scalar.copy(outsb[:, sl], psd)
                else:
                    nc.vector.tensor_copy(outsb[:, sl], psd)
                if ci == 1:
                    nc.sync.dma_start(
                        out=oB[st * 128:(st + 1) * 128, 0:1024],
                        in_=outsb[:, 0:1024])
            nc.sync.dma_start(
                out=oB[st * 128:(st + 1) * 128, 1024:2048],
                in_=outsb[:, 1024:2048])
    return nc


def _rms_rinv(x):
    v = np.mean(np.square(x, dtype=np.float64), axis=-1)
    return (1.0 / np.sqrt(v + EPS)).astype(np.float32)


def _prep_attn_inputs(xnT_bf, sid0, pos0, ln1_w, w_q, w_k, w_v, w_o):
    bf = ml_dtypes.bfloat16
    scale = D ** -0.5
    inv_freq = 1.0 / (THETA ** (np.arange(0, D, 2, dtype=np.float64) / D))
    ang = inv_freq[:, None] * pos0[None, :].astype(np.float64)  # [64, S]
    cosT = np.concatenate([np.cos(ang), np.cos(ang)], 0).astype(bf)
    sn = np.sin(ang)
    sinT = np.concatenate([-sn, sn], 0).astype(bf)
    # m01 tiles [k(part) p, (h, q256) f]: same-sid AND causal, 0/1 in bf16,
    # one [128, 512] tile per (c8, key block), causal-skipped order.
    tiles = []
    ff = np.arange(512) % 256
    for c8 in range(NC8):
        qab = c8 * 256 + ff                                    # [512]
        for b in range(2 * c8 + 2):
            kab = b * 128 + np.arange(128)                     # [128]
            m = ((sid0[kab][:, None] == sid0[qab][None, :])
                 & (kab[:, None] <= qab[None, :]))
            tiles.append(m)
    m01 = np.stack(tiles).astype(bf)                           # [72, 128, 512]

    wq_eff = ((w_q * ln1_w[None, :]).T * scale).astype(np.float32)
    wk_eff = (w_k * ln1_w[None, :]).T.astype(np.float32)
    wv_eff = (w_v * ln1_w[None, :]).T.astype(np.float32)
    woT = w_o.T.astype(np.float32)                             # [16*128, H]

    in_maps = []
    for c in range(NC):
        wq_c = wq_eff[:, c * QH * D:(c + 1) * QH * D]
        wq_t = np.ascontiguousarray(
            wq_c.reshape(NHC, 128, QH * D).transpose(1, 0, 2)).astype(bf)
        wk_c = wk_eff[:, c * D:(c + 1) * D]
        wk_t = np.ascontiguousarray(
            wk_c.reshape(NHC, 128, D).transpose(1, 0, 2)).astype(bf)
        wv_c = wv_eff[:, c * D:(c + 1) * D]
        wv_t = np.ascontiguousarray(
            wv_c.reshape(NHC, 128, D).transpose(1, 0, 2)).astype(bf)
        wo_c = woT[c * QH * D:(c + 1) * QH * D, :]             # [QH*D, H]
        wo_t = np.ascontiguousarray(
            wo_c.reshape(QH, 128, H).transpose(1, 0, 2)).astype(bf)
        in_maps.append({
            "xnT": xnT_bf, "wq": wq_t, "wk": wk_t, "wv": wv_t, "wo": wo_t,
            "cosT": cosT, "sinT": sinT, "m01": m01,
        })
    return in_maps


def _prep_mlp_inputs(ynT_bf, ln2_w, w_gate, w_up, w_down):
    bf = ml_dtypes.bfloat16
    wg_eff = (w_gate * ln2_w[None, :]).T.astype(np.float32)   # [H, INTER]
    wu_eff = (w_up * ln2_w[None, :]).T.astype(np.float32)
    wdT = w_down.T.astype(np.float32)                         # [INTER, H]
    in_maps = []
    isz = INTER // NC
    for c in range(NC):
        wg_c = wg_eff[:, c * isz:(c + 1) * isz]               # [H, 1024]
        wg_t = np.ascontiguousarray(
            wg_c.reshape(NHC, 128, MI, 128).transpose(2, 1, 0, 3)).astype(bf)
        wu_c = wu_eff[:, c * isz:(c + 1) * isz]
        wu_t = np.ascontiguousarray(
            wu_c.reshape(NHC, 128, MI, 128).transpose(2, 1, 0, 3)).astype(bf)
        wd_c = wdT[c * isz:(c + 1) * isz, :]                  # [1024, H]
        wd_t = np.ascontiguousarray(
            wd_c.reshape(MI, 128, H).transpose(1, 0, 2)).astype(bf)
        in_maps.append({"ynT": ynT_bf, "wg": wg_t, "wu": wu_t, "wd": wd_t})
    return in_maps


_cache = {}


def _get_nc(name, builder):
    if name not in _cache:
        _cache[name] = builder()
    return _cache[name]


def run(inputs, trace=False):
    bf = ml_dtypes.bfloat16
    hs0 = np.ascontiguousarray(
        np.asarray(inputs["hidden_states"], np.float32)[0])
    sid0 = np.asarray(inputs["sid"], np.int32)[0]
    pos0 = np.asarray(inputs["position_ids"], np.int32)[0]
    ln1 = np.asarray(inputs["ln1_w"], np.float32)
    ln2 = np.asarray(inputs["ln2_w"], np.float32)
    w_q = np.asarray(inputs["w_q"], np.float32)
    w_k = np.asarray(inputs["w_k"], np.float32)
    w_v = np.asarray(inputs["w_v"], np.float32)
    w_o = np.asarray(inputs["w_o"], np.float32)
    w_gate = np.asarray(inputs["w_gate"], np.float32)
    w_up = np.asarray(inputs["w_up"], np.float32)
    w_down = np.asarray(inputs["w_down"], np.float32)

    exec_times = []

    xnT_bf = np.ascontiguousarray(
        (hs0 * _rms_rinv(hs0)[:, None]).T).astype(bf)
    ncA = _get_nc("attn", build_attn)
    inA = _prep_attn_inputs(xnT_bf, sid0, pos0, ln1, w_q, w_k, w_v, w_o)
    resA = run_bass_kernel_spmd(ncA, inA, core_ids=list(range(NC)),
                                trace=trace)
    exec_times.append(resA.exec_time_ns)
    run.last_results = [resA]
    h0 = hs0 + np.sum(
        np.stack([np.asarray(r["oA"], np.float32) for r in resA.results]),
        axis=0, dtype=np.float32)

    ynT_bf = np.ascontiguousarray(
        (h0 * _rms_rinv(h0)[:, None]).T).astype(bf)
    ncB = _get_nc("mlp", build_mlp)
    inB = _prep_mlp_inputs(ynT_bf, ln2, w_gate, w_up, w_down)
    resB = run_bass_kernel_spmd(ncB, inB, core_ids=list(range(NC)),
                                trace=trace)
    exec_times.append(resB.exec_time_ns)
    run.last_results.append(resB)
    out = h0 + np.sum(
        np.stack([np.asarray(r["oB"], np.float32) for r in resB.results]),
        axis=0, dtype=np.float32)
    return out[None].astype(np.float32), exec_times


def kernel(**inputs):
    out, _ = run(inputs, trace=False)
    return out

